# revision 7
# baseline (speedup 1.0000x reference)
"""Bass/Trainium2 kernel for batched attention-score softmax.

Reference computation (B=32, S=4096, H=512):
    energy = einsum('bsh,oh->bso', encoder_outputs, W_attn) + b_attn
    scores = einsum('bso,bo->bs', energy, hidden[0])
    out    = softmax(scores, axis=1)[:, None, :]

Algebraic restructuring used here (exact, up to fp reassociation):
    scores[b,s] = enc[b,s,:] . (W_attn^T @ h[b]) + (b_attn . h[b])
The bias term is constant over s, so it cancels in the softmax and is
dropped. Precomputing v[b] = W_attn^T h[b] turns the huge [B*S,H]x[H,H]
matmul into a batched matvec, making the kernel HBM-bound on streaming
encoder_outputs (256 MB).

Sharding: data-parallel over batch B across 8 NeuronCores (4 batches per
core); W_attn replicated. No collectives needed; host gathers per-core
outputs.
"""

import numpy as np

import concourse.bacc as bacc
import concourse.tile as tile
from concourse import mybir
from concourse.bass_utils import run_bass_kernel_spmd
from concourse.masks import make_identity

P = 128            # SBUF partitions
H = 512            # hidden dim
S = 4096           # sequence length
B = 32             # global batch
NCORES = 8
BB = B // NCORES   # batches per core
HC = H // P        # h-chunks of 128
F = 8              # s-tiles per DMA chunk
ND = S // (P * F)  # DMA chunks per batch
NT = S // P        # s-tiles (score columns) per batch
FP32 = mybir.dt.float32
F16 = mybir.dt.float16
KG = 4             # s-tiles per chunk whose multiply goes to GpSimd

_nc_cache = None


def build_nc():
    nc = bacc.Bacc()
    hidden = nc.declare_dram_parameter("hidden", [BB, H], FP32, isOutput=False)
    enc = nc.declare_dram_parameter(
        "encoder_outputs", [BB, S, H], FP32, isOutput=False
    )
    W = nc.declare_dram_parameter("W_attn", [H, H], FP32, isOutput=False)
    out = nc.declare_dram_parameter("out", [BB, S], FP32, isOutput=True)

    with tile.TileContext(nc) as tc:
        with (
            tc.tile_pool(name="singles", bufs=1) as singles,
            tc.tile_pool(name="enc_pool", bufs=3) as enc_pool,
            tc.tile_pool(name="vb", bufs=BB) as vb_pool,
            tc.tile_pool(name="sc", bufs=2) as sc_pool,
            tc.tile_pool(name="sm", bufs=2) as sm_pool,
            tc.tile_pool(name="prodp", bufs=2) as prod_pool,
            tc.tile_pool(name="outp", bufs=2) as out_pool,
            tc.tile_pool(name="ps_v", bufs=2, space="PSUM") as ps_v,
            tc.tile_pool(name="ps_small", bufs=2, space="PSUM") as ps_small,
            tc.tile_pool(name="ps_t", bufs=2, space="PSUM") as ps_t,
        ):
            # --- constants / weights ---
            W_sb = singles.tile([P, HC, H], FP32)
            nc.sync.dma_start(
                out=W_sb[:], in_=W[:, :].rearrange("(c p) n -> p c n", p=P)
            )
            hT = singles.tile([P, HC, BB], FP32)
            for c in range(HC):
                nc.sync.dma_start(
                    out=hT[:, c, :],
                    in_=hidden[:, c * P : (c + 1) * P].rearrange("b p -> p b"),
                )
            ones128 = singles.tile([P, P], FP32)
            nc.vector.memset(ones128[:], 1.0)
            identity = singles.tile([P, P], FP32)
            make_identity(nc, identity[:])
            ones_col = singles.tile([P, 1], FP32)
            nc.vector.memset(ones_col[:], 1.0)
            neg_ones_row = singles.tile([1, P], FP32)
            nc.vector.memset(neg_ones_row[:], -1.0)
            ones_row = singles.tile([1, P], FP32)
            nc.vector.memset(ones_row[:], 1.0)

            # --- v[b] = W^T h[b], broadcast across partitions: [P, H] ---
            v_sbs = []
            for b in range(BB):
                v_ps = ps_v.tile([P, H], FP32, tag="v_ps")
                for c in range(HC):
                    h_bc = sm_pool.tile([P, P], FP32, tag="h_bc")
                    nc.vector.tensor_scalar_mul(
                        h_bc[:], ones128[:], hT[:, c, b : b + 1]
                    )
                    nc.tensor.matmul(
                        v_ps[:],
                        h_bc[:],
                        W_sb[:, c, :],
                        start=(c == 0),
                        stop=(c == HC - 1),
                    )
                v_sb = vb_pool.tile([P, H], FP32, tag="v_sb")
                nc.vector.tensor_copy(v_sb[:], v_ps[:])
                v_sbs.append(v_sb)

            for b in range(BB):
                # scores[p, t] = enc[b, t*128+p, :] . v[b]
                scores = sc_pool.tile([P, NT], FP32, tag="scores")
                for d in range(ND):
                    enc_t = enc_pool.tile([P, F, H], FP32, tag="enc_t")
                    s0 = d * P * F
                    nc.sync.dma_start(
                        out=enc_t[:],
                        in_=enc[b, s0 : s0 + P * F, :].rearrange(
                            "(f p) n -> p f n", p=P
                        ),
                    )
                    prod = prod_pool.tile([P, F, H], F16, tag="prod")
                    # split the multiply between GpSimd and Vector; the
                    # bf16 product lets the reduce run in packed mode
                    nc.gpsimd.tensor_mul(
                        prod[:, :KG, :],
                        enc_t[:, :KG, :],
                        v_sbs[b][:, None, :].broadcast_to([P, KG, H]),
                    )
                    nc.vector.tensor_mul(
                        prod[:, KG:, :],
                        enc_t[:, KG:, :],
                        v_sbs[b][:, None, :].broadcast_to([P, F - KG, H]),
                    )
                    nc.vector.tensor_reduce(
                        out=scores[:, d * F : (d + 1) * F],
                        in_=prod[:],
                        axis=mybir.AxisListType.X,
                        op=mybir.AluOpType.add,
                    )

                # --- softmax over all 4096 scores of batch b ---
                m_col = sm_pool.tile([P, 1], FP32, tag="m_col")
                nc.vector.tensor_reduce(
                    out=m_col[:],
                    in_=scores[:],
                    axis=mybir.AxisListType.X,
                    op=mybir.AluOpType.max,
                )
                mT_ps = ps_small.tile([1, P], FP32, tag="ps_small")
                nc.tensor.transpose(mT_ps[:], m_col[:], identity[:])
                gmax = sm_pool.tile([1, 1], FP32, tag="gmax")
                nc.vector.tensor_reduce(
                    out=gmax[:],
                    in_=mT_ps[:],
                    axis=mybir.AxisListType.X,
                    op=mybir.AluOpType.max,
                )
                ngmax_ps = ps_small.tile([P, 1], FP32, tag="ps_small")
                nc.tensor.matmul(
                    ngmax_ps[:], neg_ones_row[:], gmax[:], start=True, stop=True
                )
                ngmax = sm_pool.tile([P, 1], FP32, tag="ngmax")
                nc.vector.tensor_copy(ngmax[:], ngmax_ps[:])

                exp_sb = sm_pool.tile([P, NT], FP32, tag="exp_sb")
                rowsum = sm_pool.tile([P, 1], FP32, tag="rowsum")
                nc.scalar.activation(
                    out=exp_sb[:],
                    in_=scores[:],
                    func=mybir.ActivationFunctionType.Exp,
                    bias=ngmax[:],
                    scale=1.0,
                    accum_out=rowsum[:],
                )
                tot_ps = ps_small.tile([1, 1], FP32, tag="ps_small")
                nc.tensor.matmul(
                    tot_ps[:], rowsum[:], ones_col[:], start=True, stop=True
                )
                rtot = sm_pool.tile([1, 1], FP32, tag="rtot")
                nc.vector.reciprocal(rtot[:], tot_ps[:])
                rtot_bc_ps = ps_small.tile([P, 1], FP32, tag="ps_small")
                nc.tensor.matmul(
                    rtot_bc_ps[:], ones_row[:], rtot[:], start=True, stop=True
                )
                rtot_bc = sm_pool.tile([P, 1], FP32, tag="rtot_bc")
                nc.vector.tensor_copy(rtot_bc[:], rtot_bc_ps[:])
                norm_sb = sm_pool.tile([P, NT], FP32, tag="norm_sb")
                nc.vector.tensor_scalar_mul(norm_sb[:], exp_sb[:], rtot_bc[:])

                # transpose [P, NT] -> [NT, P] so the output DMA is contiguous
                eT_ps = ps_t.tile([NT, P], FP32, tag="eT")
                nc.tensor.transpose(eT_ps[:], norm_sb[:], identity[:])
                out_sb = out_pool.tile([NT, P], FP32, tag="out_sb")
                nc.vector.tensor_copy(out_sb[:], eT_ps[:])
                nc.sync.dma_start(
                    out=out[b].rearrange("(t p) -> t p", p=P), in_=out_sb[:]
                )
    nc.compile()
    return nc


def get_nc():
    global _nc_cache
    if _nc_cache is None:
        _nc_cache = build_nc()
    return _nc_cache


def kernel(hidden, encoder_outputs, W_attn, b_attn=None, **_unused):
    """Full inputs in, full output out; shards over 8 NeuronCores inside.

    b_attn shifts every score of a batch equally, so it cancels in the
    softmax and is not sent to the device.
    """
    hidden = np.asarray(hidden, dtype=np.float32)
    encoder_outputs = np.asarray(encoder_outputs, dtype=np.float32)
    W_attn = np.asarray(W_attn, dtype=np.float32)

    nc = get_nc()
    h2 = hidden[0]  # [B, H]
    in_maps = []
    for i in range(NCORES):
        sl = slice(i * BB, (i + 1) * BB)
        in_maps.append(
            {
                "hidden": np.ascontiguousarray(h2[sl]),
                "encoder_outputs": np.ascontiguousarray(encoder_outputs[sl]),
                "W_attn": np.ascontiguousarray(W_attn),
            }
        )
    res = run_bass_kernel_spmd(nc, in_maps, core_ids=list(range(NCORES)))
    parts = [res.results[i]["out"] for i in range(NCORES)]
    full = np.concatenate(parts, axis=0)  # [B, S]
    return full[:, None, :].astype(np.float32)


# revision 8
# speedup vs baseline: 1.2277x; 1.2277x over previous
"""Bass/Trainium2 kernel for batched attention-score softmax.

Reference computation (B=32, S=4096, H=512):
    energy = einsum('bsh,oh->bso', encoder_outputs, W_attn) + b_attn
    scores = einsum('bso,bo->bs', energy, hidden[0])
    out    = softmax(scores, axis=1)[:, None, :]

Algebraic restructuring (exact up to fp reassociation):
    scores[b,s] = enc[b,s,:] . (W_attn^T @ h[b]) + (b_attn . h[b])
The bias term is constant over s, so it cancels in the softmax and is
dropped. Precomputing v[b] = W_attn^T h[b] turns the huge [B*S,H]x[H,H]
matmul into a batched matvec, making the kernel HBM-bound on streaming
encoder_outputs (256 MB).

Sharding: data-parallel over batch B across 8 NeuronCores (4 batches
per core); W_attn replicated; host gathers per-core outputs. No
collectives needed.

Engine budget per core (~16K rows x 512): streaming enc is ~97us of DMA
at the ~358 GB/s per-core HBM limit. The multiply is split between
Vector and GpSimd, the per-row reduction between Vector (tensor_reduce)
and Scalar (activation Copy with accum_out), so no compute engine
exceeds the DMA floor.
"""

import numpy as np

import concourse.bacc as bacc
import concourse.tile as tile
from concourse import mybir
from concourse.bass_utils import run_bass_kernel_spmd
from concourse.masks import make_identity

P = 128            # SBUF partitions
H = 512            # hidden dim
S = 4096           # sequence length
B = 32             # global batch
NCORES = 8
BB = B // NCORES   # batches per core
HC = H // P        # h-chunks of 128
F = 8              # s-tiles per DMA chunk
ND = S // (P * F)  # DMA chunks per batch
NT = S // P        # s-tiles (score columns) per batch
FP32 = mybir.dt.float32
KG = 2             # s-tiles per chunk multiplied on GpSimd (rest Vector)
KA = 4             # s-tiles per chunk reduced on Scalar/ACT (rest Vector)

_nc_cache = None


def build_nc():
    nc = bacc.Bacc()
    hidden = nc.declare_dram_parameter("hidden", [BB, H], FP32, isOutput=False)
    enc = nc.declare_dram_parameter(
        "encoder_outputs", [BB, S, H], FP32, isOutput=False
    )
    W = nc.declare_dram_parameter("W_attn", [H, H], FP32, isOutput=False)
    out = nc.declare_dram_parameter("out", [BB, S], FP32, isOutput=True)

    with tile.TileContext(nc) as tc:
        with (
            tc.tile_pool(name="singles", bufs=1) as singles,
            tc.tile_pool(name="enc_pool", bufs=4) as enc_pool,
            tc.tile_pool(name="vb", bufs=BB) as vb_pool,
            tc.tile_pool(name="sc", bufs=2) as sc_pool,
            tc.tile_pool(name="sm", bufs=2) as sm_pool,
            tc.tile_pool(name="prodp", bufs=2) as prod_pool,
            tc.tile_pool(name="outp", bufs=2) as out_pool,
            tc.tile_pool(name="ps_v", bufs=2, space="PSUM") as ps_v,
            tc.tile_pool(name="ps_small", bufs=2, space="PSUM") as ps_small,
            tc.tile_pool(name="ps_t", bufs=2, space="PSUM") as ps_t,
        ):
            # --- constants / weights (aux DMAs go on the scalar HWDGE
            # ring so the sync ring carries only the enc stream) ---
            W_sb = singles.tile([P, HC, H], FP32)
            nc.scalar.dma_start(
                out=W_sb[:], in_=W[:, :].rearrange("(c p) n -> p c n", p=P)
            )
            hT = singles.tile([P, HC, BB], FP32)
            for c in range(HC):
                nc.scalar.dma_start(
                    out=hT[:, c, :],
                    in_=hidden[:, c * P : (c + 1) * P].rearrange("b p -> p b"),
                )
            ones128 = singles.tile([P, P], FP32)
            nc.vector.memset(ones128[:], 1.0)
            identity = singles.tile([P, P], FP32)
            make_identity(nc, identity[:])
            ones_col = singles.tile([P, 1], FP32)
            nc.vector.memset(ones_col[:], 1.0)
            neg_ones_row = singles.tile([1, P], FP32)
            nc.vector.memset(neg_ones_row[:], -1.0)
            ones_row = singles.tile([1, P], FP32)
            nc.vector.memset(ones_row[:], 1.0)

            # --- v[b] = W^T h[b], broadcast across partitions: [P, H] ---
            v_sbs = []
            for b in range(BB):
                v_ps = ps_v.tile([P, H], FP32, tag="v_ps")
                for c in range(HC):
                    # h_bc[p, m] = h[b, c*128+p] for all m (ACT copy with
                    # per-partition scale)
                    h_bc = sm_pool.tile([P, P], FP32, tag="h_bc")
                    nc.scalar.mul(h_bc[:], ones128[:], hT[:, c, b : b + 1])
                    nc.tensor.matmul(
                        v_ps[:],
                        h_bc[:],
                        W_sb[:, c, :],
                        start=(c == 0),
                        stop=(c == HC - 1),
                    )
                v_sb = vb_pool.tile([P, H], FP32, tag="v_sb")
                nc.scalar.copy(v_sb[:], v_ps[:])
                v_sbs.append(v_sb)

            for b in range(BB):
                # scores[p, t] = enc[b, t*128+p, :] . v[b]
                scores = sc_pool.tile([P, NT], FP32, tag="scores")
                for d in range(ND):
                    enc_t = enc_pool.tile([P, F, H], FP32, tag="enc_t")
                    s0 = d * P * F
                    nc.sync.dma_start(
                        out=enc_t[:],
                        in_=enc[b, s0 : s0 + P * F, :].rearrange(
                            "(f p) n -> p f n", p=P
                        ),
                    )
                    prod = prod_pool.tile([P, F, H], FP32, tag="prod")
                    vb = v_sbs[b]
                    # multiply: sub-tiles [0:KG] on GpSimd, rest on Vector
                    nc.gpsimd.tensor_mul(
                        prod[:, :KG, :],
                        enc_t[:, :KG, :],
                        vb[:, None, :].broadcast_to([P, KG, H]),
                    )
                    nc.vector.tensor_mul(
                        prod[:, KG:, :],
                        enc_t[:, KG:, :],
                        vb[:, None, :].broadcast_to([P, F - KG, H]),
                    )
                    # reduce: sub-tiles [0:KA] on ACT (Copy + accum_out),
                    # rest on Vector as one 3D tensor_reduce
                    for t in range(KA):
                        nc.scalar.activation(
                            out=prod[:, t, :],
                            in_=prod[:, t, :],
                            func=mybir.ActivationFunctionType.Copy,
                            accum_out=scores[:, d * F + t : d * F + t + 1],
                        )
                    nc.vector.tensor_reduce(
                        out=scores[:, d * F + KA : (d + 1) * F],
                        in_=prod[:, KA:, :],
                        axis=mybir.AxisListType.X,
                        op=mybir.AluOpType.add,
                    )

                # --- softmax over all 4096 scores of batch b ---
                m_col = sm_pool.tile([P, 1], FP32, tag="m_col")
                nc.vector.tensor_reduce(
                    out=m_col[:],
                    in_=scores[:],
                    axis=mybir.AxisListType.X,
                    op=mybir.AluOpType.max,
                )
                mT_ps = ps_small.tile([1, P], FP32, tag="ps_small")
                nc.tensor.transpose(mT_ps[:], m_col[:], identity[:])
                gmax = sm_pool.tile([1, 1], FP32, tag="gmax")
                nc.vector.tensor_reduce(
                    out=gmax[:],
                    in_=mT_ps[:],
                    axis=mybir.AxisListType.X,
                    op=mybir.AluOpType.max,
                )
                ngmax_ps = ps_small.tile([P, 1], FP32, tag="ps_small")
                nc.tensor.matmul(
                    ngmax_ps[:], neg_ones_row[:], gmax[:], start=True, stop=True
                )
                ngmax = sm_pool.tile([P, 1], FP32, tag="ngmax")
                nc.vector.tensor_copy(ngmax[:], ngmax_ps[:])

                exp_sb = sm_pool.tile([P, NT], FP32, tag="exp_sb")
                rowsum = sm_pool.tile([P, 1], FP32, tag="rowsum")
                nc.scalar.activation(
                    out=exp_sb[:],
                    in_=scores[:],
                    func=mybir.ActivationFunctionType.Exp,
                    bias=ngmax[:],
                    scale=1.0,
                    accum_out=rowsum[:],
                )
                tot_ps = ps_small.tile([1, 1], FP32, tag="ps_small")
                nc.tensor.matmul(
                    tot_ps[:], rowsum[:], ones_col[:], start=True, stop=True
                )
                rtot = sm_pool.tile([1, 1], FP32, tag="rtot")
                nc.vector.reciprocal(rtot[:], tot_ps[:])
                rtot_bc_ps = ps_small.tile([P, 1], FP32, tag="ps_small")
                nc.tensor.matmul(
                    rtot_bc_ps[:], ones_row[:], rtot[:], start=True, stop=True
                )
                rtot_bc = sm_pool.tile([P, 1], FP32, tag="rtot_bc")
                nc.vector.tensor_copy(rtot_bc[:], rtot_bc_ps[:])
                norm_sb = sm_pool.tile([P, NT], FP32, tag="norm_sb")
                nc.vector.tensor_scalar_mul(norm_sb[:], exp_sb[:], rtot_bc[:])

                # transpose [P, NT] -> [NT, P] so the output DMA is contiguous
                eT_ps = ps_t.tile([NT, P], FP32, tag="eT")
                nc.tensor.transpose(eT_ps[:], norm_sb[:], identity[:])
                out_sb = out_pool.tile([NT, P], FP32, tag="out_sb")
                nc.vector.tensor_copy(out_sb[:], eT_ps[:])
                nc.scalar.dma_start(
                    out=out[b].rearrange("(t p) -> t p", p=P), in_=out_sb[:]
                )
    nc.compile()
    return nc


def get_nc():
    global _nc_cache
    if _nc_cache is None:
        _nc_cache = build_nc()
    return _nc_cache


def kernel(hidden, encoder_outputs, W_attn, b_attn=None, **_unused):
    """Full inputs in, full output out; shards over 8 NeuronCores inside.

    b_attn shifts every score of a batch equally, so it cancels in the
    softmax and is not sent to the device.
    """
    hidden = np.asarray(hidden, dtype=np.float32)
    encoder_outputs = np.asarray(encoder_outputs, dtype=np.float32)
    W_attn = np.asarray(W_attn, dtype=np.float32)

    nc = get_nc()
    h2 = hidden[0]  # [B, H]
    in_maps = []
    for i in range(NCORES):
        sl = slice(i * BB, (i + 1) * BB)
        in_maps.append(
            {
                "hidden": np.ascontiguousarray(h2[sl]),
                "encoder_outputs": np.ascontiguousarray(encoder_outputs[sl]),
                "W_attn": np.ascontiguousarray(W_attn),
            }
        )
    res = run_bass_kernel_spmd(nc, in_maps, core_ids=list(range(NCORES)))
    parts = [res.results[i]["out"] for i in range(NCORES)]
    full = np.concatenate(parts, axis=0)  # [B, S]
    return full[:, None, :].astype(np.float32)
